# revision 53
# baseline (speedup 1.0000x reference)
"""Trainium2 Bass kernel for a 16-head causal MHA layer.

Problem: x:[2,2048,1024] f32, wq/wk/wv/wo:[1024,1024] f32 (Linear-style
[out,in] weights), causal softmax attention with 16 heads of dim 64.

Sharding across the 8 NeuronCores: 2-way data parallel over batch x
4-way tensor parallel over heads.  Core c handles batch c//4 and the 4
heads 4*(c%4) .. 4*(c%4)+3 (feature slice of 256 rows of wq/wk/wv and
256 columns of wo).  Each core produces a partial [2048,1024] output
(its 4 heads' contribution, already projected through its wo slice);
the host sums the 4 partials per batch.

Device dataflow (all matmul inputs fp16, fp32 PSUM accumulation):
  - host uploads x already transposed per batch: xT [1024, 2048] fp16
  - qT/kT = W @ xT in [feat, token] layout; v in [token, feat] layout,
    with a constant-1 column appended per head (v|1)
  - scoresT[k,q] = kT_h.T-block @ qT_h (64-dim contraction), exp on ACT
    straight out of PSUM (no max subtraction: |scores/8| < ~4 so exp is
    safe in fp32/fp16), causal mask applied only on diagonal blocks via
    a precomputed 0/1 mask multiply
  - out_unnorm.T | l = (v|1).T-block @ expT accumulated over k blocks
    (the appended ones-column yields the softmax denominator l for free)
  - 1/l via a DRAM-roundtrip transpose to [128,x] + DVE reciprocal,
    broadcast back across partitions, multiply into out_unnorm.T
  - y = outT.T @ woT accumulated over the 256-dim feature slice
"""

import numpy as np

S = 2048          # sequence length (one batch per core)
D = 1024          # model dim
HL = 4            # heads handled per core
DH = 64           # head dim
F = HL * DH       # 256 local features
DC = D // 128     # 8 d_model chunks of 128
FC = F // 128     # 2 feature chunks of 128
NT = S // 128     # 16 token tiles
NQ = S // 512     # 4 query chunks of 512

_CACHE = {}


def _build_program(dbg=False):
    key = ("nc", dbg)
    if key in _CACHE:
        return _CACHE[key]

    import concourse.bacc as bacc
    import concourse.bass as bass
    import concourse.mybir as mybir
    import concourse.tile as tile

    f16 = mybir.dt.float16
    f32 = mybir.dt.float32
    Exp = mybir.ActivationFunctionType.Exp

    nc = bacc.Bacc("TRN2", target_bir_lowering=False, debug=False)

    xT_d = nc.dram_tensor("xT", [DC, 128, S], f16, kind="ExternalInput")
    wqT_d = nc.dram_tensor("wqT", [DC, 128, F], f16, kind="ExternalInput")
    wkT_d = nc.dram_tensor("wkT", [DC, 128, F], f16, kind="ExternalInput")
    wvT_d = nc.dram_tensor("wvT", [DC, 128, F], f16, kind="ExternalInput")
    woT_d = nc.dram_tensor("woT", [FC, 128, D], f16, kind="ExternalInput")
    mask_d = nc.dram_tensor("mask", [128, 896], f16, kind="ExternalInput")
    ident_d = nc.dram_tensor("ident", [128, 128], f16, kind="ExternalInput")
    y_d = nc.dram_tensor("y", [S, D], f16, kind="ExternalOutput")
    if dbg:
        qT_dbg = nc.dram_tensor("qT_dbg", [128, FC, S], f16, kind="ExternalOutput")
        kT_dbg = nc.dram_tensor("kT_dbg", [128, FC, S], f16, kind="ExternalOutput")
        v_dbg = nc.dram_tensor("v_dbg", [128, NT, HL, DH + 1], f16, kind="ExternalOutput")
        outT_dbg = nc.dram_tensor("outT_dbg", [128, FC, S], f16, kind="ExternalOutput")
        l_dbg = nc.dram_tensor("l_dbg", [HL * S], f32, kind="ExternalOutput")

    with tile.TileContext(nc) as tc:
        with tc.tile_pool(name="const", bufs=1) as cpool, \
             tc.tile_pool(name="dscr", bufs=1,
                          space=bass.MemorySpace.DRAM) as dpool:
            l_dram = dpool.tile([HL * S], f32)
            xT = cpool.tile([128, DC, S], f16)
            wq = cpool.tile([128, DC, F], f16)
            wk = cpool.tile([128, DC, F], f16)
            wv = cpool.tile([128, DC, F], f16)
            wo = cpool.tile([128, FC, D], f16)
            mask = cpool.tile([128, 896], f16)
            ident = cpool.tile([128, 128], f16)
            qT = cpool.tile([128, FC, S], f16)
            kT = cpool.tile([128, FC, S], f16)
            v = cpool.tile([128, NT, HL, DH + 1], f16)
            outT = cpool.tile([128, FC, S], f16)
            l_row = cpool.tile([1, HL * S], f32)
            recip_row = cpool.tile([1, HL * S], f32)
            recip16_row = cpool.tile([1, HL * S], f16)
            ones1 = cpool.tile([1, DH], f16)
            lT = cpool.tile([128, HL * NT], f32)
            recipT = cpool.tile([128, HL * NT], f32)
            recipT16 = cpool.tile([128, HL * NT], f16)

            # loads: the wire is ~110GB/s per queue x 3 queues, so wave-1
            # (everything attention chunk 0 needs: wq+wk+wv per-dc slices,
            # x tokens 0:512, mask -- 2.8MB) is spread evenly over
            # sync/scalar/gpsimd and issued before anything else.  Weights
            # load per-dc with no rearrange (cheap descriptors, 512B lines).
            # wq/wk on the fast HWDGE queues (gpsimd SWDGE transfers start
            # ~5us late -- fine for wv, fatal for q/k); x split into three
            # token ranges per dc so each attention chunk's tokens land
            # with several us of slack before their projection fillers run
            # (JIT-tight arrivals force serial LDWEIGHTS on every matmul).
            nc.scalar.dma_start(mask[:], mask_d[:])
            for dc in range(DC):
                nc.sync.dma_start(wq[:, dc, :], wqT_d[dc])
                nc.scalar.dma_start(wk[:, dc, :], wkT_d[dc])
                nc.gpsimd.dma_start(wv[:, dc, :], wvT_d[dc])
            for dc in range(DC):
                q = (nc.sync, nc.scalar)[dc % 2]
                q.dma_start(xT[:, dc, 0:512], xT_d[dc][:, 0:512])
            for dc in range(DC):
                q = (nc.sync, nc.scalar)[dc % 2]
                q.dma_start(xT[:, dc, 512:1024], xT_d[dc][:, 512:1024])
            for dc in range(DC):
                q = (nc.sync, nc.scalar)[dc % 2]
                q.dma_start(xT[:, dc, 1024:2048], xT_d[dc][:, 1024:2048])
            nc.gpsimd.dma_start(ident[:], ident_d[:])
            for fcw in range(FC):
                nc.gpsimd.dma_start(wo[:, fcw, :], woT_d[fcw])
            # preload the exp table set (~2.7us) after all DMA issues so it
            # costs the scalar queue nothing; the first real activation
            # then doesn't pay the table load
            nc.scalar.activation(recip_row[0:1, 0:4], ones1[0:1, 0:4], Exp)

            # ---- attention + normalize + output projection -------------
            # qc-major: all heads for query-chunk qc, then (lagged by one
            # chunk so every dependency is long ready) the softmax
            # normalization and wo projection for chunk qc-1.  The wo/bc
            # matmuls fill the PE bubbles of the exp-bound attention loop.
            with tc.tile_pool(name="sc_ps", bufs=2,
                              space=bass.MemorySpace.PSUM) as scp, \
                 tc.tile_pool(name="av_ps", bufs=2,
                              space=bass.MemorySpace.PSUM) as avp, \
                 tc.tile_pool(name="ybc_ps", bufs=2,
                              space=bass.MemorySpace.PSUM) as ybcp, \
                 tc.tile_pool(name="p_sb", bufs=6) as ppool, \
                 tc.tile_pool(name="y_sb", bufs=8) as ysb_pool:

                # Projection groups are split into two ~0.4us filler units
                # (dc 0-3 / dc 4-7) so the filler granularity matches the PE
                # bubbles inside the attention blocks.  The psum tile spans
                # the two parts; parts are always queued adjacently so at
                # most one split tile is pending at a time (ybc pool bufs=2).
                _split_ps = {}

                def proj_qk_part(w_sb, dstT, fc, t5, part):
                    key = ("qk", dstT is kT, fc, t5)
                    if part == 0:
                        ps = ybcp.tile([128, 512], f32, tag="ybc",
                                       name=f"ps_{key[1]}_{t5}_{fc}")
                        _split_ps[key] = ps
                    else:
                        ps = _split_ps.pop(key)
                    for dc in (range(0, 4) if part == 0 else range(4, DC)):
                        nc.tensor.matmul(
                            ps[:],
                            w_sb[:, dc, fc * 128:(fc + 1) * 128],
                            xT[:, dc, t5 * 512:(t5 + 1) * 512],
                            start=(dc == 0), stop=(dc == DC - 1))
                    if part == 1:
                        nc.vector.tensor_copy(
                            dstT[:, fc, t5 * 512:(t5 + 1) * 512], ps[:])

                def proj_qk_group(w_sb, dstT, fc, t5):
                    proj_qk_part(w_sb, dstT, fc, t5, 0)
                    proj_qk_part(w_sb, dstT, fc, t5, 1)

                def proj_v_part(tt, part):
                    key = ("v", tt)
                    if part == 0:
                        psv = ybcp.tile([128, F], f32, tag="ybc",
                                        name=f"psv_{tt}")
                        _split_ps[key] = psv
                    else:
                        psv = _split_ps.pop(key)
                    for dc in (range(0, 4) if part == 0 else range(4, DC)):
                        nc.tensor.matmul(
                            psv[:],
                            xT[:, dc, tt * 128:(tt + 1) * 128],
                            wv[:, dc, :],
                            start=(dc == 0), stop=(dc == DC - 1))
                    if part == 1:
                        nc.vector.tensor_copy(
                            v[:, tt, :, 0:DH],
                            psv.rearrange("p (h d) -> p h d", h=HL))

                def proj_v(tts):
                    for tt in tts:
                        proj_v_part(tt, 0)
                        proj_v_part(tt, 1)

                import collections
                fillers = collections.deque()

                # HAM warmup: dummy matmuls during the input-load window so
                # the PE clock-gate is at 8/8 when real work arrives.  Few
                # enough not to delay the first projection matmuls (the PE
                # queue is strictly in-order).
                warm = ppool.tile([128, 128], f16, tag="warm", bufs=1)
                # warm memset first so the HAM warmup matmuls can start the
                # moment the engines clear the runtime preamble
                nc.vector.memset(warm[:], 1.0)
                nc.vector.memset(v[:], 1.0)   # ones cols for the denom trick
                nc.vector.memset(ones1[:], 1.0)
                warm_ctr = [0]

                def warm_mm(n=1, w=256):
                    # fresh rotating psum tile per call so the warm tile's
                    # lifetime never pins a ybc pool slot across the body
                    warm_ctr[0] += 1
                    wps = ybcp.tile([128, w], f32, tag="ybc",
                                    name=f"warm_ps_{warm_ctr[0]}")
                    for _ in range(n):
                        nc.tensor.matmul(
                            wps[:], warm[:],
                            warm[:, 0:1].to_broadcast((128, w)),
                            start=True, stop=True)

                warm_mm(16)

                def run_filler(n):
                    for _ in range(n):
                        if fillers:
                            fillers.popleft()()

                def att_hc(qc, hc):
                    if True:
                        avs = []
                        for hp2 in range(2):
                            av = avp.tile([DH + 1, 512], f32, tag="av",
                                          name=f"av_{hc}_{qc}_{hp2}")
                            avs.append(av)
                        for g in range(qc + 1):
                            diag = (g == qc)
                            for half in range(2):
                                # (offset, width) of each k-block's valid
                                # q-span inside the p tile; diagonal blocks
                                # are clipped to q >= k_block_start
                                if diag:
                                    rs = [2 * half, 2 * half + 1]
                                    spans = [(128 * r, 512 - 128 * r)
                                             for r in rs]
                                else:
                                    spans = [(0, 512), (0, 512)]
                                offs = [0, spans[0][1]]
                                scs = []
                                for hp2 in range(2):
                                    sc = scp.tile([128, 1024], f32, tag="sc",
                                                  name=f"sc_{hc}_{qc}_{g}_{half}_{hp2}")
                                    scs.append(sc)
                                for r2 in range(2):
                                    kb = 4 * g + 2 * half + r2
                                    qo, w = spans[r2]
                                    for hp2 in range(2):
                                        hp = hp2 * 64
                                        nc.tensor.matmul(
                                            scs[hp2][:, offs[r2]:offs[r2] + w],
                                            kT[hp:hp + 64, hc,
                                               kb * 128:(kb + 1) * 128],
                                            qT[hp:hp + 64, hc,
                                               qc * 512 + qo:(qc + 1) * 512],
                                            start=True, stop=True,
                                            tile_position=(hp, 0))
                                width = offs[1] + spans[1][1]
                                # issue both heads' exp (and diag masks)
                                # first, then fill the PE bubble while ACT
                                # works with one filler unit per AV group
                                p_sbs = []
                                for hp2 in range(2):
                                    p_sb = ppool.tile([128, 1024], f16,
                                                      tag=f"p{hp2}",
                                                      name=f"p_{hc}_{qc}_{g}_{half}_{hp2}")
                                    p_sbs.append(p_sb)
                                    nc.scalar.activation(
                                        p_sb[:, 0:width],
                                        scs[hp2][:, 0:width], Exp)
                                    if diag:
                                        # only the first 128 columns of a
                                        # clipped block straddle the diagonal
                                        for r2 in range(2):
                                            nc.vector.tensor_mul(
                                                p_sb[:, offs[r2]:offs[r2] + 128],
                                                p_sb[:, offs[r2]:offs[r2] + 128],
                                                mask[:, 384:512])
                                for hp2 in range(2):
                                    run_filler(1)
                                    h = hc * 2 + hp2
                                    for r2 in range(2):
                                        kb = 4 * g + 2 * half + r2
                                        qo, w = spans[r2]
                                        nc.tensor.matmul(
                                            avs[hp2][:, qo:512],
                                            v[:, kb, h, :],
                                            p_sbs[hp2][:, offs[r2]:offs[r2] + w],
                                            start=(kb == 0),
                                            stop=(kb == 4 * qc + 3))
                        # denominator rows first (they head the DMA roundtrip
                        # critical path), split across two queues; the last
                        # chunk's copies go to Scalar (idle once its exps are
                        # done) so they don't queue behind the outT CASTs
                        last = (qc == NQ - 1 and hc == 1)
                        for hp2 in range(2):
                            h = hc * 2 + hp2
                            seg = slice(h * S + qc * 512,
                                        h * S + (qc + 1) * 512)
                            if last:
                                nc.scalar.copy(l_row[0:1, seg],
                                               avs[hp2][DH:DH + 1, :])
                            else:
                                nc.vector.tensor_copy(l_row[0:1, seg],
                                                      avs[hp2][DH:DH + 1, :])
                            if qc < NQ - 1:
                                # body chunks: transpose roundtrip via DMA
                                # (latency hidden by the half-chunk lag; the
                                # [128,4]-layout reciprocal is far cheaper
                                # on DVE than a [1,512] one)
                                nc.sync.dma_start(l_dram[seg],
                                                  l_row[0:1, seg])
                                nc.sync.dma_start(
                                    lT[:, h * NT + 4 * qc:
                                       h * NT + 4 * qc + 4],
                                    l_dram[seg].rearrange("(t p) -> p t",
                                                          p=128))
                        for hp2 in range(2):
                            hp = hp2 * 64
                            if last:
                                # keep the DVE free for the tail's
                                # reciprocal chain; ACT is idle here
                                nc.scalar.copy(
                                    outT[hp:hp + 64, hc,
                                         qc * 512:(qc + 1) * 512],
                                    avs[hp2][0:DH, :])
                            else:
                                nc.vector.tensor_copy(
                                    outT[hp:hp + 64, hc,
                                         qc * 512:(qc + 1) * 512],
                                    avs[hp2][0:DH, :])

                def norm_half(qc, hc, hp2):
                    h = hc * 2 + hp2
                    hp = hp2 * 64
                    bc = ybcp.tile([64, 512], f32, tag="ybc",
                                   name=f"bc_{h}_{qc}")
                    if qc == NQ - 1:
                        # last chunk: no-DMA path — fast approximate
                        # reciprocal straight on the [1,512] denominator
                        # row, broadcast across the 64 dh partitions with a
                        # single K=1 matmul ones[1,64].T @ recip_row[1,512].
                        # (Too DVE-heavy for the body, but the DVE is idle
                        # here and it kills the exposed roundtrip latency.)
                        seg = slice(h * S + qc * 512, h * S + (qc + 1) * 512)
                        nc.vector.reciprocal_approx_fast(
                            recip_row[0:1, seg], l_row[0:1, seg])
                        nc.vector.tensor_copy(
                            recip16_row[0:1, seg], recip_row[0:1, seg])
                        nc.tensor.matmul(
                            bc[:], ones1[0:1, :], recip16_row[0:1, seg],
                            start=True, stop=True)
                    else:
                        # body: 1/l on the [q-partition] transposed copy,
                        # broadcast over the 64 dh rows with K=128 matmuls
                        # against the identity (tiny DVE footprint)
                        c = slice(h * NT + 4 * qc, h * NT + 4 * qc + 4)
                        nc.vector.reciprocal(recipT[:, c], lT[:, c])
                        nc.vector.tensor_copy(recipT16[:, c], recipT[:, c])
                        for t4 in range(4):
                            col = h * NT + 4 * qc + t4
                            nc.tensor.matmul(
                                bc[:, t4 * 128:(t4 + 1) * 128],
                                recipT16[:, col:col + 1]
                                .to_broadcast((128, DH)),
                                ident[:],
                                start=True, stop=True)
                    nc.vector.tensor_mul(
                        outT[hp:hp + 64, hc, qc * 512:(qc + 1) * 512],
                        outT[hp:hp + 64, hc, qc * 512:(qc + 1) * 512],
                        bc[:])

                def wo_tile(qt, oc):
                    if True:
                        if True:
                            yps = ybcp.tile([128, 512], f32, tag="ybc",
                                            name=f"yps_{qt}_{oc}")
                            for fc in range(FC):
                                nc.tensor.matmul(
                                    yps[:],
                                    outT[:, fc, qt * 128:(qt + 1) * 128],
                                    wo[:, fc, oc * 512:(oc + 1) * 512],
                                    start=(fc == 0), stop=(fc == FC - 1))
                            ysb = ysb_pool.tile([128, 512], f16, tag="ysb",
                                                name=f"ysb_{qt}_{oc}")
                            # final chunk: alternate the psum->sbuf cast
                            # between Vector and Scalar so 8 casts don't
                            # serialize on one engine at the very end
                            if qt >= 4 * (NQ - 1) and oc == 1:
                                nc.scalar.copy(ysb[:], yps[:])
                            else:
                                nc.vector.tensor_copy(ysb[:], yps[:])
                            # y stores: sync hw queue (gpsimd software-DGE
                            # stores measured slower overall); final chunk
                            # alternates sync/scalar so the last 1MB drains
                            # 2-wide while both queues are idle
                            if qt >= 4 * (NQ - 1):
                                q = (nc.sync, nc.scalar)[(qt * 2 + oc) % 2]
                            else:
                                q = nc.sync
                            q.dma_start(
                                y_d[qt * 128:(qt + 1) * 128,
                                    oc * 512:(oc + 1) * 512],
                                ysb[:])

                # fc0 projections + v first so attention starts earliest;
                # fc1 q/k groups run as fillers inside att(0,0)
                for w_sb, dstT in ((wq, qT), (wk, kT)):
                    proj_qk_group(w_sb, dstT, 0, 0)
                proj_v(range(0, 4))
                for w_sb, dstT in ((wq, qT), (wk, kT)):
                    for part in range(2):
                        fillers.append(
                            lambda w=w_sb, d=dstT, p=part:
                            proj_qk_part(w, d, 1, 0, p))

                # Schedule (half-chunk lag): norm(qc,0) runs as a filler late
                # inside att(qc,1); norm(qc,1) + wo(qc) run inside att(qc+1,0).
                # Keeps every lT DMA roundtrip hidden by a full att phase and
                # leaves only norm(3,1)+wo(3) as the (short, HAM-kept-warm)
                # tail.  All filler units are ~0.4-0.8us of PE work so they
                # match the per-AV-group bubbles inside the blocks.
                for qc in range(NQ):
                    if qc >= 1:
                        for hp2 in range(2):
                            fillers.append(
                                lambda q=qc - 1, p=hp2: norm_half(q, 1, p))
                        for qt in range(4 * (qc - 1), 4 * qc):
                            for oc in range(2):
                                fillers.append(
                                    lambda a=qt, b=oc: wo_tile(a, b))
                    att_hc(qc, 0)
                    # keep 2 units back so the next att call's first blocks
                    # have filler work to cover their exp-pipeline refill
                    # (safe: the tail of the deque here is norm/wo units,
                    # which have no dependency on the next chunk's q/k)
                    run_filler(max(0, len(fillers) - (2 if qc >= 1 else 0)))
                    if qc + 1 < NQ:
                        for w_sb, dstT in ((wq, qT), (wk, kT)):
                            for fc in range(FC):
                                for part in range(2):
                                    fillers.append(
                                        lambda w=w_sb, d=dstT, f=fc,
                                        t=qc + 1, p=part:
                                        proj_qk_part(w, d, f, t, p))
                        for tt in range(4 * (qc + 1), 4 * (qc + 2)):
                            for part in range(2):
                                fillers.append(
                                    lambda t=tt, p=part: proj_v_part(t, p))
                    for hp2 in range(2):
                        fillers.append(
                            lambda q=qc, p=hp2: norm_half(q, 0, p))
                    att_hc(qc, 1)
                    # qc=0 must fully drain (qk(1) units pend and att(1,0)
                    # scores would deadlock behind them) and so must the
                    # last chunk (no more run_filler calls after the loop);
                    # middle chunks keep 2 norm units back for the next
                    # chunk-start bubble
                    keep = 2 if 1 <= qc < NQ - 1 else 0
                    run_filler(max(0, len(fillers) - keep))
                # tail: bridge the last lT roundtrip (~2us) with just enough
                # warm matmuls to hold the PE clock-gate at 8/8 without
                # delaying the final norm+wo work behind them
                warm_mm(8)
                norm_half(NQ - 1, 1, 0)
                norm_half(NQ - 1, 1, 1)
                for qt in range(4 * (NQ - 1), 4 * NQ):
                    for oc in range(2):
                        wo_tile(qt, oc)

            if dbg:
                nc.sync.dma_start(qT_dbg[:], qT[:])
                nc.sync.dma_start(kT_dbg[:], kT[:])
                nc.sync.dma_start(v_dbg[:], v[:])
                nc.sync.dma_start(outT_dbg[:], outT[:])
                nc.sync.dma_start(l_dbg[:], l_row[0:1, :])

    nc.compile()

    from concourse.bass_interp import get_hw_module
    nc.m = get_hw_module(nc.m)

    _CACHE[key] = nc
    return nc


def _make_mask():
    # mask[p, j] = 1 where (j - p) >= 384; slices of width 512 at offset
    # 384-128*r give the causal mask for a diagonal block at relative
    # position r (k block kb = 4*qc + r vs the 512-wide q chunk qc)
    j = np.arange(896)[None, :]
    p = np.arange(128)[:, None]
    return ((j - p) >= 384).astype(np.float16)


def kernel(x, wq, wk, wv, wo):
    x = np.asarray(x, dtype=np.float32)
    wq = np.asarray(wq, dtype=np.float32)
    wk = np.asarray(wk, dtype=np.float32)
    wv = np.asarray(wv, dtype=np.float32)
    wo = np.asarray(wo, dtype=np.float32)

    from concourse import bass_utils

    nc = _build_program()
    mask = _make_mask()

    in_maps = []
    for c in range(8):
        b = c // 4
        hg = c % 4
        fs = slice(hg * F, (hg + 1) * F)
        xT = np.ascontiguousarray(x[b].T).astype(np.float16).reshape(DC, 128, S)
        wqT = np.ascontiguousarray((wq[fs, :] * 0.125).T).astype(np.float16)
        wkT = np.ascontiguousarray(wk[fs, :].T).astype(np.float16)
        wvT = np.ascontiguousarray(wv[fs, :].T).astype(np.float16)
        woT = np.ascontiguousarray(wo[:, fs].T).astype(np.float16)
        in_maps.append({
            "xT": xT,
            "wqT": wqT.reshape(DC, 128, F),
            "wkT": wkT.reshape(DC, 128, F),
            "wvT": wvT.reshape(DC, 128, F),
            "woT": woT.reshape(FC, 128, D),
            "mask": mask,
            "ident": np.eye(128, dtype=np.float16),
        })

    res = bass_utils.run_bass_kernel_spmd(nc, in_maps, core_ids=list(range(8)))
    ys = [res.results[c]["y"].astype(np.float32) for c in range(8)]
    out = np.stack([ys[0] + ys[1] + ys[2] + ys[3],
                    ys[4] + ys[5] + ys[6] + ys[7]])
    return out



# revision 54
# speedup vs baseline: 1.0061x; 1.0061x over previous
"""Trainium2 Bass kernel for a 16-head causal MHA layer.

Problem: x:[2,2048,1024] f32, wq/wk/wv/wo:[1024,1024] f32 (Linear-style
[out,in] weights), causal softmax attention with 16 heads of dim 64.

Sharding across the 8 NeuronCores: 2-way data parallel over batch x
4-way tensor parallel over heads.  Core c handles batch c//4 and the 4
heads 4*(c%4) .. 4*(c%4)+3 (feature slice of 256 rows of wq/wk/wv and
256 columns of wo).  Each core produces a partial [2048,1024] output
(its 4 heads' contribution, already projected through its wo slice);
the host sums the 4 partials per batch.

Device dataflow (all matmul inputs fp16, fp32 PSUM accumulation):
  - host uploads x already transposed per batch: xT [1024, 2048] fp16
  - qT/kT = W @ xT in [feat, token] layout; v in [token, feat] layout,
    with a constant-1 column appended per head (v|1)
  - scoresT[k,q] = kT_h.T-block @ qT_h (64-dim contraction), exp on ACT
    straight out of PSUM (no max subtraction: |scores/8| < ~4 so exp is
    safe in fp32/fp16), causal mask applied only on diagonal blocks via
    a precomputed 0/1 mask multiply
  - out_unnorm.T | l = (v|1).T-block @ expT accumulated over k blocks
    (the appended ones-column yields the softmax denominator l for free)
  - 1/l via a DRAM-roundtrip transpose to [128,x] + DVE reciprocal,
    broadcast back across partitions, multiply into out_unnorm.T
  - y = outT.T @ woT accumulated over the 256-dim feature slice
"""

import numpy as np

S = 2048          # sequence length (one batch per core)
D = 1024          # model dim
HL = 4            # heads handled per core
DH = 64           # head dim
F = HL * DH       # 256 local features
DC = D // 128     # 8 d_model chunks of 128
FC = F // 128     # 2 feature chunks of 128
NT = S // 128     # 16 token tiles
NQ = S // 512     # 4 query chunks of 512

_CACHE = {}


def _build_program(dbg=False):
    key = ("nc", dbg)
    if key in _CACHE:
        return _CACHE[key]

    import concourse.bacc as bacc
    import concourse.bass as bass
    import concourse.mybir as mybir
    import concourse.tile as tile

    f16 = mybir.dt.float16
    f32 = mybir.dt.float32
    Exp = mybir.ActivationFunctionType.Exp

    nc = bacc.Bacc("TRN2", target_bir_lowering=False, debug=False)

    xT_d = nc.dram_tensor("xT", [DC, 128, S], f16, kind="ExternalInput")
    wqT_d = nc.dram_tensor("wqT", [DC, 128, F], f16, kind="ExternalInput")
    wkT_d = nc.dram_tensor("wkT", [DC, 128, F], f16, kind="ExternalInput")
    wvT_d = nc.dram_tensor("wvT", [DC, 128, F], f16, kind="ExternalInput")
    woT_d = nc.dram_tensor("woT", [FC, 128, D], f16, kind="ExternalInput")
    mask_d = nc.dram_tensor("mask", [128, 896], f16, kind="ExternalInput")
    ident_d = nc.dram_tensor("ident", [128, 128], f16, kind="ExternalInput")
    y_d = nc.dram_tensor("y", [S, D], f16, kind="ExternalOutput")
    if dbg:
        qT_dbg = nc.dram_tensor("qT_dbg", [128, FC, S], f16, kind="ExternalOutput")
        kT_dbg = nc.dram_tensor("kT_dbg", [128, FC, S], f16, kind="ExternalOutput")
        v_dbg = nc.dram_tensor("v_dbg", [128, NT, HL, DH + 1], f16, kind="ExternalOutput")
        outT_dbg = nc.dram_tensor("outT_dbg", [128, FC, S], f16, kind="ExternalOutput")
        l_dbg = nc.dram_tensor("l_dbg", [HL * S], f32, kind="ExternalOutput")

    with tile.TileContext(nc) as tc:
        with tc.tile_pool(name="const", bufs=1) as cpool, \
             tc.tile_pool(name="dscr", bufs=1,
                          space=bass.MemorySpace.DRAM) as dpool:
            l_dram = dpool.tile([HL * S], f32)
            xT = cpool.tile([128, DC, S], f16)
            wq = cpool.tile([128, DC, F], f16)
            wk = cpool.tile([128, DC, F], f16)
            wv = cpool.tile([128, DC, F], f16)
            wo = cpool.tile([128, FC, D], f16)
            mask = cpool.tile([128, 896], f16)
            ident = cpool.tile([128, 128], f16)
            qT = cpool.tile([128, FC, S], f16)
            kT = cpool.tile([128, FC, S], f16)
            v = cpool.tile([128, NT, HL, DH + 1], f16)
            outT = cpool.tile([128, FC, S], f16)
            l_row = cpool.tile([1, HL * S], f32)
            recip_row = cpool.tile([1, HL * S], f32)
            recip16_row = cpool.tile([1, HL * S], f16)
            ones1 = cpool.tile([1, DH], f16)
            lT = cpool.tile([128, HL * NT], f32)
            recipT = cpool.tile([128, HL * NT], f32)
            recipT16 = cpool.tile([128, HL * NT], f16)

            # loads: the wire is ~110GB/s per queue x 3 queues, so wave-1
            # (everything attention chunk 0 needs: wq+wk+wv per-dc slices,
            # x tokens 0:512, mask -- 2.8MB) is spread evenly over
            # sync/scalar/gpsimd and issued before anything else.  Weights
            # load per-dc with no rearrange (cheap descriptors, 512B lines).
            nc.scalar.dma_start(mask[:], mask_d[:])
            for dc in range(DC):
                nc.gpsimd.dma_start(wq[:, dc, :], wqT_d[dc])
                q = (nc.sync, nc.scalar)[dc % 2]
                q.dma_start(xT[:, dc, 0:512], xT_d[dc][:, 0:512])
            for dc in range(DC):
                q = (nc.sync, nc.scalar)[(dc + 1) % 2]
                q.dma_start(wk[:, dc, :], wkT_d[dc])
                nc.gpsimd.dma_start(wv[:, dc, :], wvT_d[dc])
            # wave 2: remaining x tokens, wo, ident
            for dc in range(DC):
                q = (nc.sync, nc.scalar)[dc % 2]
                q.dma_start(xT[:, dc, 512:2048], xT_d[dc][:, 512:2048])
            nc.gpsimd.dma_start(ident[:], ident_d[:])
            for fcw in range(FC):
                nc.gpsimd.dma_start(wo[:, fcw, :], woT_d[fcw])
            # preload the exp table set (~2.7us) after all DMA issues so it
            # costs the scalar queue nothing; the first real activation
            # then doesn't pay the table load
            nc.scalar.activation(recip_row[0:1, 0:4], ones1[0:1, 0:4], Exp)

            # ---- attention + normalize + output projection -------------
            # qc-major: all heads for query-chunk qc, then (lagged by one
            # chunk so every dependency is long ready) the softmax
            # normalization and wo projection for chunk qc-1.  The wo/bc
            # matmuls fill the PE bubbles of the exp-bound attention loop.
            with tc.tile_pool(name="sc_ps", bufs=2,
                              space=bass.MemorySpace.PSUM) as scp, \
                 tc.tile_pool(name="av_ps", bufs=2,
                              space=bass.MemorySpace.PSUM) as avp, \
                 tc.tile_pool(name="ybc_ps", bufs=2,
                              space=bass.MemorySpace.PSUM) as ybcp, \
                 tc.tile_pool(name="p_sb", bufs=6) as ppool, \
                 tc.tile_pool(name="y_sb", bufs=8) as ysb_pool:

                # Projection groups are split into two ~0.4us filler units
                # (dc 0-3 / dc 4-7) so the filler granularity matches the PE
                # bubbles inside the attention blocks.  The psum tile spans
                # the two parts; parts are always queued adjacently so at
                # most one split tile is pending at a time (ybc pool bufs=2).
                _split_ps = {}

                def proj_qk_part(w_sb, dstT, fc, t5, part):
                    key = ("qk", dstT is kT, fc, t5)
                    if part == 0:
                        ps = ybcp.tile([128, 512], f32, tag="ybc",
                                       name=f"ps_{key[1]}_{t5}_{fc}")
                        _split_ps[key] = ps
                    else:
                        ps = _split_ps.pop(key)
                    for dc in (range(0, 4) if part == 0 else range(4, DC)):
                        nc.tensor.matmul(
                            ps[:],
                            w_sb[:, dc, fc * 128:(fc + 1) * 128],
                            xT[:, dc, t5 * 512:(t5 + 1) * 512],
                            start=(dc == 0), stop=(dc == DC - 1))
                    if part == 1:
                        nc.vector.tensor_copy(
                            dstT[:, fc, t5 * 512:(t5 + 1) * 512], ps[:])

                def proj_qk_group(w_sb, dstT, fc, t5):
                    proj_qk_part(w_sb, dstT, fc, t5, 0)
                    proj_qk_part(w_sb, dstT, fc, t5, 1)

                def proj_v_part(tt, part):
                    key = ("v", tt)
                    if part == 0:
                        psv = ybcp.tile([128, F], f32, tag="ybc",
                                        name=f"psv_{tt}")
                        _split_ps[key] = psv
                    else:
                        psv = _split_ps.pop(key)
                    for dc in (range(0, 4) if part == 0 else range(4, DC)):
                        nc.tensor.matmul(
                            psv[:],
                            xT[:, dc, tt * 128:(tt + 1) * 128],
                            wv[:, dc, :],
                            start=(dc == 0), stop=(dc == DC - 1))
                    if part == 1:
                        nc.vector.tensor_copy(
                            v[:, tt, :, 0:DH],
                            psv.rearrange("p (h d) -> p h d", h=HL))

                def proj_v(tts):
                    for tt in tts:
                        proj_v_part(tt, 0)
                        proj_v_part(tt, 1)

                import collections
                fillers = collections.deque()

                # HAM warmup: dummy matmuls during the input-load window so
                # the PE clock-gate is at 8/8 when real work arrives.  Few
                # enough not to delay the first projection matmuls (the PE
                # queue is strictly in-order).
                warm = ppool.tile([128, 128], f16, tag="warm", bufs=1)
                # warm memset first so the HAM warmup matmuls can start the
                # moment the engines clear the runtime preamble
                nc.vector.memset(warm[:], 1.0)
                nc.vector.memset(v[:], 1.0)   # ones cols for the denom trick
                nc.vector.memset(ones1[:], 1.0)
                warm_ctr = [0]

                def warm_mm(n=1, w=256):
                    # fresh rotating psum tile per call so the warm tile's
                    # lifetime never pins a ybc pool slot across the body
                    warm_ctr[0] += 1
                    wps = ybcp.tile([128, w], f32, tag="ybc",
                                    name=f"warm_ps_{warm_ctr[0]}")
                    for _ in range(n):
                        nc.tensor.matmul(
                            wps[:], warm[:],
                            warm[:, 0:1].to_broadcast((128, w)),
                            start=True, stop=True)

                warm_mm(16)

                def run_filler(n):
                    for _ in range(n):
                        if fillers:
                            fillers.popleft()()

                def att_hc(qc, hc):
                    if True:
                        avs = []
                        for hp2 in range(2):
                            av = avp.tile([DH + 1, 512], f32, tag="av",
                                          name=f"av_{hc}_{qc}_{hp2}")
                            avs.append(av)
                        for g in range(qc + 1):
                            diag = (g == qc)
                            for half in range(2):
                                # (offset, width) of each k-block's valid
                                # q-span inside the p tile; diagonal blocks
                                # are clipped to q >= k_block_start
                                if diag:
                                    rs = [2 * half, 2 * half + 1]
                                    spans = [(128 * r, 512 - 128 * r)
                                             for r in rs]
                                else:
                                    spans = [(0, 512), (0, 512)]
                                offs = [0, spans[0][1]]
                                scs = []
                                for hp2 in range(2):
                                    sc = scp.tile([128, 1024], f32, tag="sc",
                                                  name=f"sc_{hc}_{qc}_{g}_{half}_{hp2}")
                                    scs.append(sc)
                                for r2 in range(2):
                                    kb = 4 * g + 2 * half + r2
                                    qo, w = spans[r2]
                                    for hp2 in range(2):
                                        hp = hp2 * 64
                                        nc.tensor.matmul(
                                            scs[hp2][:, offs[r2]:offs[r2] + w],
                                            kT[hp:hp + 64, hc,
                                               kb * 128:(kb + 1) * 128],
                                            qT[hp:hp + 64, hc,
                                               qc * 512 + qo:(qc + 1) * 512],
                                            start=True, stop=True,
                                            tile_position=(hp, 0))
                                width = offs[1] + spans[1][1]
                                # issue both heads' exp (and diag masks)
                                # first, then fill the PE bubble while ACT
                                # works with one filler unit per AV group
                                p_sbs = []
                                for hp2 in range(2):
                                    p_sb = ppool.tile([128, 1024], f16,
                                                      tag=f"p{hp2}",
                                                      name=f"p_{hc}_{qc}_{g}_{half}_{hp2}")
                                    p_sbs.append(p_sb)
                                    nc.scalar.activation(
                                        p_sb[:, 0:width],
                                        scs[hp2][:, 0:width], Exp)
                                    if diag:
                                        # only the first 128 columns of a
                                        # clipped block straddle the diagonal
                                        for r2 in range(2):
                                            nc.vector.tensor_mul(
                                                p_sb[:, offs[r2]:offs[r2] + 128],
                                                p_sb[:, offs[r2]:offs[r2] + 128],
                                                mask[:, 384:512])
                                for hp2 in range(2):
                                    run_filler(1)
                                    h = hc * 2 + hp2
                                    for r2 in range(2):
                                        kb = 4 * g + 2 * half + r2
                                        qo, w = spans[r2]
                                        nc.tensor.matmul(
                                            avs[hp2][:, qo:512],
                                            v[:, kb, h, :],
                                            p_sbs[hp2][:, offs[r2]:offs[r2] + w],
                                            start=(kb == 0),
                                            stop=(kb == 4 * qc + 3))
                        # denominator rows first (they head the DMA roundtrip
                        # critical path), split across two queues; the last
                        # chunk's copies go to Scalar (idle once its exps are
                        # done) so they don't queue behind the outT CASTs
                        last = (qc == NQ - 1 and hc == 1)
                        for hp2 in range(2):
                            h = hc * 2 + hp2
                            seg = slice(h * S + qc * 512,
                                        h * S + (qc + 1) * 512)
                            if last:
                                nc.scalar.copy(l_row[0:1, seg],
                                               avs[hp2][DH:DH + 1, :])
                            else:
                                nc.vector.tensor_copy(l_row[0:1, seg],
                                                      avs[hp2][DH:DH + 1, :])
                            if qc < NQ - 1:
                                # body chunks: transpose roundtrip via DMA
                                # (latency hidden by the half-chunk lag; the
                                # [128,4]-layout reciprocal is far cheaper
                                # on DVE than a [1,512] one)
                                nc.sync.dma_start(l_dram[seg],
                                                  l_row[0:1, seg])
                                nc.sync.dma_start(
                                    lT[:, h * NT + 4 * qc:
                                       h * NT + 4 * qc + 4],
                                    l_dram[seg].rearrange("(t p) -> p t",
                                                          p=128))
                        for hp2 in range(2):
                            hp = hp2 * 64
                            if last:
                                # keep the DVE free for the tail's
                                # reciprocal chain; ACT is idle here
                                nc.scalar.copy(
                                    outT[hp:hp + 64, hc,
                                         qc * 512:(qc + 1) * 512],
                                    avs[hp2][0:DH, :])
                            else:
                                nc.vector.tensor_copy(
                                    outT[hp:hp + 64, hc,
                                         qc * 512:(qc + 1) * 512],
                                    avs[hp2][0:DH, :])

                def norm_half(qc, hc, hp2):
                    h = hc * 2 + hp2
                    hp = hp2 * 64
                    bc = ybcp.tile([64, 512], f32, tag="ybc",
                                   name=f"bc_{h}_{qc}")
                    if qc == NQ - 1:
                        # last chunk: no-DMA path — fast approximate
                        # reciprocal straight on the [1,512] denominator
                        # row, broadcast across the 64 dh partitions with a
                        # single K=1 matmul ones[1,64].T @ recip_row[1,512].
                        # (Too DVE-heavy for the body, but the DVE is idle
                        # here and it kills the exposed roundtrip latency.)
                        seg = slice(h * S + qc * 512, h * S + (qc + 1) * 512)
                        nc.vector.reciprocal_approx_fast(
                            recip_row[0:1, seg], l_row[0:1, seg])
                        nc.vector.tensor_copy(
                            recip16_row[0:1, seg], recip_row[0:1, seg])
                        nc.tensor.matmul(
                            bc[:], ones1[0:1, :], recip16_row[0:1, seg],
                            start=True, stop=True)
                    else:
                        # body: 1/l on the [q-partition] transposed copy,
                        # broadcast over the 64 dh rows with K=128 matmuls
                        # against the identity (tiny DVE footprint)
                        c = slice(h * NT + 4 * qc, h * NT + 4 * qc + 4)
                        nc.vector.reciprocal(recipT[:, c], lT[:, c])
                        nc.vector.tensor_copy(recipT16[:, c], recipT[:, c])
                        for t4 in range(4):
                            col = h * NT + 4 * qc + t4
                            nc.tensor.matmul(
                                bc[:, t4 * 128:(t4 + 1) * 128],
                                recipT16[:, col:col + 1]
                                .to_broadcast((128, DH)),
                                ident[:],
                                start=True, stop=True)
                    nc.vector.tensor_mul(
                        outT[hp:hp + 64, hc, qc * 512:(qc + 1) * 512],
                        outT[hp:hp + 64, hc, qc * 512:(qc + 1) * 512],
                        bc[:])

                def wo_tile(qt, oc):
                    if True:
                        if True:
                            yps = ybcp.tile([128, 512], f32, tag="ybc",
                                            name=f"yps_{qt}_{oc}")
                            for fc in range(FC):
                                nc.tensor.matmul(
                                    yps[:],
                                    outT[:, fc, qt * 128:(qt + 1) * 128],
                                    wo[:, fc, oc * 512:(oc + 1) * 512],
                                    start=(fc == 0), stop=(fc == FC - 1))
                            ysb = ysb_pool.tile([128, 512], f16, tag="ysb",
                                                name=f"ysb_{qt}_{oc}")
                            # final chunk: alternate the psum->sbuf cast
                            # between Vector and Scalar so 8 casts don't
                            # serialize on one engine at the very end
                            if qt >= 4 * (NQ - 1) and oc == 1:
                                nc.scalar.copy(ysb[:], yps[:])
                            else:
                                nc.vector.tensor_copy(ysb[:], yps[:])
                            # y stores: sync hw queue (gpsimd software-DGE
                            # stores measured slower overall); final chunk
                            # alternates sync/scalar so the last 1MB drains
                            # 2-wide while both queues are idle
                            if qt >= 4 * (NQ - 1):
                                q = (nc.sync, nc.scalar)[(qt * 2 + oc) % 2]
                            else:
                                q = nc.sync
                            q.dma_start(
                                y_d[qt * 128:(qt + 1) * 128,
                                    oc * 512:(oc + 1) * 512],
                                ysb[:])

                # fc0 projections + v first so attention starts earliest;
                # fc1 q/k groups run as fillers inside att(0,0)
                for w_sb, dstT in ((wq, qT), (wk, kT)):
                    proj_qk_group(w_sb, dstT, 0, 0)
                proj_v(range(0, 4))
                for w_sb, dstT in ((wq, qT), (wk, kT)):
                    for part in range(2):
                        fillers.append(
                            lambda w=w_sb, d=dstT, p=part:
                            proj_qk_part(w, d, 1, 0, p))

                # Schedule (half-chunk lag): norm(qc,0) runs as a filler late
                # inside att(qc,1); norm(qc,1) + wo(qc) run inside att(qc+1,0).
                # Keeps every lT DMA roundtrip hidden by a full att phase and
                # leaves only norm(3,1)+wo(3) as the (short, HAM-kept-warm)
                # tail.  All filler units are ~0.4-0.8us of PE work so they
                # match the per-AV-group bubbles inside the blocks.
                for qc in range(NQ):
                    if qc >= 1:
                        for hp2 in range(2):
                            fillers.append(
                                lambda q=qc - 1, p=hp2: norm_half(q, 1, p))
                        for qt in range(4 * (qc - 1), 4 * qc):
                            for oc in range(2):
                                fillers.append(
                                    lambda a=qt, b=oc: wo_tile(a, b))
                    att_hc(qc, 0)
                    # keep 2 units back so the next att call's first blocks
                    # have filler work to cover their exp-pipeline refill
                    # (safe: the tail of the deque here is norm/wo units,
                    # which have no dependency on the next chunk's q/k)
                    run_filler(max(0, len(fillers) - (2 if qc >= 1 else 0)))
                    if qc + 1 < NQ:
                        for w_sb, dstT in ((wq, qT), (wk, kT)):
                            for fc in range(FC):
                                for part in range(2):
                                    fillers.append(
                                        lambda w=w_sb, d=dstT, f=fc,
                                        t=qc + 1, p=part:
                                        proj_qk_part(w, d, f, t, p))
                        for tt in range(4 * (qc + 1), 4 * (qc + 2)):
                            for part in range(2):
                                fillers.append(
                                    lambda t=tt, p=part: proj_v_part(t, p))
                    for hp2 in range(2):
                        fillers.append(
                            lambda q=qc, p=hp2: norm_half(q, 0, p))
                    att_hc(qc, 1)
                    # qc=0 must fully drain (qk(1) units pend and att(1,0)
                    # scores would deadlock behind them) and so must the
                    # last chunk (no more run_filler calls after the loop);
                    # middle chunks keep 2 norm units back for the next
                    # chunk-start bubble
                    keep = 2 if 1 <= qc < NQ - 1 else 0
                    run_filler(max(0, len(fillers) - keep))
                # tail: bridge the last lT roundtrip (~2us) with just enough
                # warm matmuls to hold the PE clock-gate at 8/8 without
                # delaying the final norm+wo work behind them
                warm_mm(8)
                norm_half(NQ - 1, 1, 0)
                norm_half(NQ - 1, 1, 1)
                for qt in range(4 * (NQ - 1), 4 * NQ):
                    for oc in range(2):
                        wo_tile(qt, oc)

            if dbg:
                nc.sync.dma_start(qT_dbg[:], qT[:])
                nc.sync.dma_start(kT_dbg[:], kT[:])
                nc.sync.dma_start(v_dbg[:], v[:])
                nc.sync.dma_start(outT_dbg[:], outT[:])
                nc.sync.dma_start(l_dbg[:], l_row[0:1, :])

    nc.compile()

    from concourse.bass_interp import get_hw_module
    nc.m = get_hw_module(nc.m)

    _CACHE[key] = nc
    return nc


def _make_mask():
    # mask[p, j] = 1 where (j - p) >= 384; slices of width 512 at offset
    # 384-128*r give the causal mask for a diagonal block at relative
    # position r (k block kb = 4*qc + r vs the 512-wide q chunk qc)
    j = np.arange(896)[None, :]
    p = np.arange(128)[:, None]
    return ((j - p) >= 384).astype(np.float16)


def kernel(x, wq, wk, wv, wo):
    x = np.asarray(x, dtype=np.float32)
    wq = np.asarray(wq, dtype=np.float32)
    wk = np.asarray(wk, dtype=np.float32)
    wv = np.asarray(wv, dtype=np.float32)
    wo = np.asarray(wo, dtype=np.float32)

    from concourse import bass_utils

    nc = _build_program()
    mask = _make_mask()

    in_maps = []
    for c in range(8):
        b = c // 4
        hg = c % 4
        fs = slice(hg * F, (hg + 1) * F)
        xT = np.ascontiguousarray(x[b].T).astype(np.float16).reshape(DC, 128, S)
        wqT = np.ascontiguousarray((wq[fs, :] * 0.125).T).astype(np.float16)
        wkT = np.ascontiguousarray(wk[fs, :].T).astype(np.float16)
        wvT = np.ascontiguousarray(wv[fs, :].T).astype(np.float16)
        woT = np.ascontiguousarray(wo[:, fs].T).astype(np.float16)
        in_maps.append({
            "xT": xT,
            "wqT": wqT.reshape(DC, 128, F),
            "wkT": wkT.reshape(DC, 128, F),
            "wvT": wvT.reshape(DC, 128, F),
            "woT": woT.reshape(FC, 128, D),
            "mask": mask,
            "ident": np.eye(128, dtype=np.float16),
        })

    res = bass_utils.run_bass_kernel_spmd(nc, in_maps, core_ids=list(range(8)))
    ys = [res.results[c]["y"].astype(np.float32) for c in range(8)]
    out = np.stack([ys[0] + ys[1] + ys[2] + ys[3],
                    ys[4] + ys[5] + ys[6] + ys[7]])
    return out



# revision 55
# speedup vs baseline: 1.0247x; 1.0184x over previous
"""Trainium2 Bass kernel for a 16-head causal MHA layer.

Problem: x:[2,2048,1024] f32, wq/wk/wv/wo:[1024,1024] f32 (Linear-style
[out,in] weights), causal softmax attention with 16 heads of dim 64.

Sharding across the 8 NeuronCores: 2-way data parallel over batch x
4-way tensor parallel over heads.  Core c handles batch c//4 and the 4
heads 4*(c%4) .. 4*(c%4)+3 (feature slice of 256 rows of wq/wk/wv and
256 columns of wo).  Each core produces a partial [2048,1024] output
(its 4 heads' contribution, already projected through its wo slice);
the host sums the 4 partials per batch.

Device dataflow (all matmul inputs fp16, fp32 PSUM accumulation):
  - host uploads x already transposed per batch: xT [1024, 2048] fp16
  - qT/kT = W @ xT in [feat, token] layout; v in [token, feat] layout,
    with a constant-1 column appended per head (v|1)
  - scoresT[k,q] = kT_h.T-block @ qT_h (64-dim contraction), exp on ACT
    straight out of PSUM (no max subtraction: |scores/8| < ~4 so exp is
    safe in fp32/fp16), causal mask applied only on diagonal blocks via
    a precomputed 0/1 mask multiply
  - out_unnorm.T | l = (v|1).T-block @ expT accumulated over k blocks
    (the appended ones-column yields the softmax denominator l for free)
  - 1/l: body chunks use a DRAM-roundtrip transpose to [128,4] + DVE
    reciprocal + identity-matmul broadcast (latency hidden by the
    half-chunk lag); the last chunk uses reciprocal_approx_fast on the
    raw [1,512] row + a K=1 ones-matmul broadcast (no DMA, so the tail
    isn't exposed to the roundtrip latency)
  - y = outT.T @ woT accumulated over the 256-dim feature slice

Scheduling: attention blocks are software-pipelined (both heads' exps
issue right after the score matmuls; one ~0.5us filler unit runs before
each head's AV group), with projections/normalization/wo for other
chunks supplying the filler units at half-chunk lag.  DMA queueing and
the measured hardware costs behind each choice are documented inline.
"""

import numpy as np

S = 2048          # sequence length (one batch per core)
D = 1024          # model dim
HL = 4            # heads handled per core
DH = 64           # head dim
F = HL * DH       # 256 local features
DC = D // 128     # 8 d_model chunks of 128
FC = F // 128     # 2 feature chunks of 128
NT = S // 128     # 16 token tiles
NQ = S // 512     # 4 query chunks of 512

_CACHE = {}


def _build_program(dbg=False):
    key = ("nc", dbg)
    if key in _CACHE:
        return _CACHE[key]

    import concourse.bacc as bacc
    import concourse.bass as bass
    import concourse.mybir as mybir
    import concourse.tile as tile

    f16 = mybir.dt.float16
    f32 = mybir.dt.float32
    Exp = mybir.ActivationFunctionType.Exp

    nc = bacc.Bacc("TRN2", target_bir_lowering=False, debug=False)

    xT_d = nc.dram_tensor("xT", [DC, 128, S], f16, kind="ExternalInput")
    wqT_d = nc.dram_tensor("wqT", [DC, 128, F], f16, kind="ExternalInput")
    wkT_d = nc.dram_tensor("wkT", [DC, 128, F], f16, kind="ExternalInput")
    wvT_d = nc.dram_tensor("wvT", [DC, 128, F], f16, kind="ExternalInput")
    woT_d = nc.dram_tensor("woT", [FC, 128, D], f16, kind="ExternalInput")
    mask_d = nc.dram_tensor("mask", [128, 896], f16, kind="ExternalInput")
    ident_d = nc.dram_tensor("ident", [128, 128], f16, kind="ExternalInput")
    y_d = nc.dram_tensor("y", [S, D], f16, kind="ExternalOutput")
    if dbg:
        qT_dbg = nc.dram_tensor("qT_dbg", [128, FC, S], f16, kind="ExternalOutput")
        kT_dbg = nc.dram_tensor("kT_dbg", [128, FC, S], f16, kind="ExternalOutput")
        v_dbg = nc.dram_tensor("v_dbg", [128, NT, HL, DH + 1], f16, kind="ExternalOutput")
        outT_dbg = nc.dram_tensor("outT_dbg", [128, FC, S], f16, kind="ExternalOutput")
        l_dbg = nc.dram_tensor("l_dbg", [HL * S], f32, kind="ExternalOutput")

    with tile.TileContext(nc) as tc:
        with tc.tile_pool(name="const", bufs=1) as cpool, \
             tc.tile_pool(name="dscr", bufs=1,
                          space=bass.MemorySpace.DRAM) as dpool:
            l_dram = dpool.tile([HL * S], f32)
            xT = cpool.tile([128, DC, S], f16)
            wq = cpool.tile([128, DC, F], f16)
            wk = cpool.tile([128, DC, F], f16)
            wv = cpool.tile([128, DC, F], f16)
            wo = cpool.tile([128, FC, D], f16)
            mask = cpool.tile([128, 896], f16)
            ident = cpool.tile([128, 128], f16)
            qT = cpool.tile([128, FC, S], f16)
            kT = cpool.tile([128, FC, S], f16)
            v = cpool.tile([128, NT, HL, DH + 1], f16)
            outT = cpool.tile([128, FC, S], f16)
            l_row = cpool.tile([1, HL * S], f32)
            recip_row = cpool.tile([1, HL * S], f32)
            recip16_row = cpool.tile([1, HL * S], f16)
            ones1 = cpool.tile([1, DH], f16)
            lT = cpool.tile([128, HL * NT], f32)
            recipT = cpool.tile([128, HL * NT], f32)
            recipT16 = cpool.tile([128, HL * NT], f16)

            # loads: the wire is ~110GB/s per queue x 3 queues, so wave-1
            # (everything attention chunk 0 needs: wq+wk+wv per-dc slices,
            # x tokens 0:512, mask -- 2.8MB) is spread evenly over
            # sync/scalar/gpsimd and issued before anything else.  Weights
            # load per-dc with no rearrange (cheap descriptors, 512B lines).
            nc.scalar.dma_start(mask[:], mask_d[:])
            for dc in range(DC):
                nc.gpsimd.dma_start(wq[:, dc, :], wqT_d[dc])
                q = (nc.sync, nc.scalar)[dc % 2]
                q.dma_start(xT[:, dc, 0:512], xT_d[dc][:, 0:512])
            for dc in range(DC):
                q = (nc.sync, nc.scalar)[(dc + 1) % 2]
                q.dma_start(wk[:, dc, :], wkT_d[dc])
                nc.gpsimd.dma_start(wv[:, dc, :], wvT_d[dc])
            # wave 2: remaining x tokens, wo, ident
            for dc in range(DC):
                q = (nc.sync, nc.scalar)[dc % 2]
                q.dma_start(xT[:, dc, 512:2048], xT_d[dc][:, 512:2048])
            nc.gpsimd.dma_start(ident[:], ident_d[:])
            for fcw in range(FC):
                nc.gpsimd.dma_start(wo[:, fcw, :], woT_d[fcw])
            # preload the exp table set (~2.7us) after all DMA issues so it
            # costs the scalar queue nothing; the first real activation
            # then doesn't pay the table load
            nc.scalar.activation(recip_row[0:1, 0:4], ones1[0:1, 0:4], Exp)

            # ---- attention + normalize + output projection -------------
            # qc-major: all heads for query-chunk qc, then (lagged by one
            # chunk so every dependency is long ready) the softmax
            # normalization and wo projection for chunk qc-1.  The wo/bc
            # matmuls fill the PE bubbles of the exp-bound attention loop.
            with tc.tile_pool(name="sc_ps", bufs=2,
                              space=bass.MemorySpace.PSUM) as scp, \
                 tc.tile_pool(name="av_ps", bufs=2,
                              space=bass.MemorySpace.PSUM) as avp, \
                 tc.tile_pool(name="ybc_ps", bufs=2,
                              space=bass.MemorySpace.PSUM) as ybcp, \
                 tc.tile_pool(name="p_sb", bufs=6) as ppool, \
                 tc.tile_pool(name="y_sb", bufs=8) as ysb_pool:

                # Projection groups are split into two ~0.4us filler units
                # (dc 0-3 / dc 4-7) so the filler granularity matches the PE
                # bubbles inside the attention blocks.  The psum tile spans
                # the two parts; parts are always queued adjacently so at
                # most one split tile is pending at a time (ybc pool bufs=2).
                _split_ps = {}

                def proj_qk_part(w_sb, dstT, fc, t5, part):
                    key = ("qk", dstT is kT, fc, t5)
                    if part == 0:
                        ps = ybcp.tile([128, 512], f32, tag="ybc",
                                       name=f"ps_{key[1]}_{t5}_{fc}")
                        _split_ps[key] = ps
                    else:
                        ps = _split_ps.pop(key)
                    for dc in (range(0, 4) if part == 0 else range(4, DC)):
                        nc.tensor.matmul(
                            ps[:],
                            w_sb[:, dc, fc * 128:(fc + 1) * 128],
                            xT[:, dc, t5 * 512:(t5 + 1) * 512],
                            start=(dc == 0), stop=(dc == DC - 1))
                    if part == 1:
                        nc.vector.tensor_copy(
                            dstT[:, fc, t5 * 512:(t5 + 1) * 512], ps[:])

                def proj_qk_group(w_sb, dstT, fc, t5):
                    proj_qk_part(w_sb, dstT, fc, t5, 0)
                    proj_qk_part(w_sb, dstT, fc, t5, 1)

                def proj_v_part(tt, part):
                    key = ("v", tt)
                    if part == 0:
                        psv = ybcp.tile([128, F], f32, tag="ybc",
                                        name=f"psv_{tt}")
                        _split_ps[key] = psv
                    else:
                        psv = _split_ps.pop(key)
                    for dc in (range(0, 4) if part == 0 else range(4, DC)):
                        nc.tensor.matmul(
                            psv[:],
                            xT[:, dc, tt * 128:(tt + 1) * 128],
                            wv[:, dc, :],
                            start=(dc == 0), stop=(dc == DC - 1))
                    if part == 1:
                        nc.vector.tensor_copy(
                            v[:, tt, :, 0:DH],
                            psv.rearrange("p (h d) -> p h d", h=HL))

                def proj_v(tts):
                    for tt in tts:
                        proj_v_part(tt, 0)
                        proj_v_part(tt, 1)

                import collections
                fillers = collections.deque()

                # HAM warmup: dummy matmuls during the input-load window so
                # the PE clock-gate is at 8/8 when real work arrives.  Few
                # enough not to delay the first projection matmuls (the PE
                # queue is strictly in-order).
                warm = ppool.tile([128, 128], f16, tag="warm", bufs=1)
                # warm memset first so the HAM warmup matmuls can start the
                # moment the engines clear the runtime preamble
                nc.vector.memset(warm[:], 1.0)
                nc.vector.memset(v[:], 1.0)   # ones cols for the denom trick
                nc.vector.memset(ones1[:], 1.0)
                warm_ctr = [0]

                def warm_mm(n=1, w=256):
                    # fresh rotating psum tile per call so the warm tile's
                    # lifetime never pins a ybc pool slot across the body
                    warm_ctr[0] += 1
                    wps = ybcp.tile([128, w], f32, tag="ybc",
                                    name=f"warm_ps_{warm_ctr[0]}")
                    for _ in range(n):
                        nc.tensor.matmul(
                            wps[:], warm[:],
                            warm[:, 0:1].to_broadcast((128, w)),
                            start=True, stop=True)

                warm_mm(16)

                def run_filler(n):
                    for _ in range(n):
                        if fillers:
                            fillers.popleft()()

                def att_hc(qc, hc):
                    if True:
                        avs = []
                        for hp2 in range(2):
                            av = avp.tile([DH + 1, 512], f32, tag="av",
                                          name=f"av_{hc}_{qc}_{hp2}")
                            avs.append(av)
                        for g in range(qc + 1):
                            diag = (g == qc)
                            for half in range(2):
                                # (offset, width) of each k-block's valid
                                # q-span inside the p tile; diagonal blocks
                                # are clipped to q >= k_block_start
                                if diag:
                                    rs = [2 * half, 2 * half + 1]
                                    spans = [(128 * r, 512 - 128 * r)
                                             for r in rs]
                                else:
                                    spans = [(0, 512), (0, 512)]
                                offs = [0, spans[0][1]]
                                scs = []
                                for hp2 in range(2):
                                    sc = scp.tile([128, 1024], f32, tag="sc",
                                                  name=f"sc_{hc}_{qc}_{g}_{half}_{hp2}")
                                    scs.append(sc)
                                for r2 in range(2):
                                    kb = 4 * g + 2 * half + r2
                                    qo, w = spans[r2]
                                    for hp2 in range(2):
                                        hp = hp2 * 64
                                        nc.tensor.matmul(
                                            scs[hp2][:, offs[r2]:offs[r2] + w],
                                            kT[hp:hp + 64, hc,
                                               kb * 128:(kb + 1) * 128],
                                            qT[hp:hp + 64, hc,
                                               qc * 512 + qo:(qc + 1) * 512],
                                            start=True, stop=True,
                                            tile_position=(hp, 0))
                                width = offs[1] + spans[1][1]
                                # issue both heads' exp (and diag masks)
                                # first, then fill the PE bubble while ACT
                                # works with one filler unit per AV group
                                p_sbs = []
                                for hp2 in range(2):
                                    p_sb = ppool.tile([128, 1024], f16,
                                                      tag=f"p{hp2}",
                                                      name=f"p_{hc}_{qc}_{g}_{half}_{hp2}")
                                    p_sbs.append(p_sb)
                                    nc.scalar.activation(
                                        p_sb[:, 0:width],
                                        scs[hp2][:, 0:width], Exp)
                                    if diag:
                                        # only the first 128 columns of a
                                        # clipped block straddle the diagonal
                                        for r2 in range(2):
                                            nc.vector.tensor_mul(
                                                p_sb[:, offs[r2]:offs[r2] + 128],
                                                p_sb[:, offs[r2]:offs[r2] + 128],
                                                mask[:, 384:512])
                                for hp2 in range(2):
                                    run_filler(1)
                                    h = hc * 2 + hp2
                                    for r2 in range(2):
                                        kb = 4 * g + 2 * half + r2
                                        qo, w = spans[r2]
                                        nc.tensor.matmul(
                                            avs[hp2][:, qo:512],
                                            v[:, kb, h, :],
                                            p_sbs[hp2][:, offs[r2]:offs[r2] + w],
                                            start=(kb == 0),
                                            stop=(kb == 4 * qc + 3))
                        # denominator rows first (they head the DMA roundtrip
                        # critical path), split across two queues; the last
                        # chunk's copies go to Scalar (idle once its exps are
                        # done) so they don't queue behind the outT CASTs
                        last = (qc == NQ - 1 and hc == 1)
                        for hp2 in range(2):
                            h = hc * 2 + hp2
                            seg = slice(h * S + qc * 512,
                                        h * S + (qc + 1) * 512)
                            if last:
                                nc.scalar.copy(l_row[0:1, seg],
                                               avs[hp2][DH:DH + 1, :])
                            else:
                                nc.vector.tensor_copy(l_row[0:1, seg],
                                                      avs[hp2][DH:DH + 1, :])
                            if qc < NQ - 1:
                                # body chunks: transpose roundtrip via DMA
                                # (latency hidden by the half-chunk lag; the
                                # [128,4]-layout reciprocal is far cheaper
                                # on DVE than a [1,512] one)
                                nc.sync.dma_start(l_dram[seg],
                                                  l_row[0:1, seg])
                                nc.sync.dma_start(
                                    lT[:, h * NT + 4 * qc:
                                       h * NT + 4 * qc + 4],
                                    l_dram[seg].rearrange("(t p) -> p t",
                                                          p=128))
                        for hp2 in range(2):
                            hp = hp2 * 64
                            if last:
                                # keep the DVE free for the tail's
                                # reciprocal chain; ACT is idle here
                                nc.scalar.copy(
                                    outT[hp:hp + 64, hc,
                                         qc * 512:(qc + 1) * 512],
                                    avs[hp2][0:DH, :])
                            else:
                                nc.vector.tensor_copy(
                                    outT[hp:hp + 64, hc,
                                         qc * 512:(qc + 1) * 512],
                                    avs[hp2][0:DH, :])

                def norm_half(qc, hc, hp2):
                    h = hc * 2 + hp2
                    hp = hp2 * 64
                    bc = ybcp.tile([64, 512], f32, tag="ybc",
                                   name=f"bc_{h}_{qc}")
                    if qc == NQ - 1:
                        # last chunk: no-DMA path — fast approximate
                        # reciprocal straight on the [1,512] denominator
                        # row, broadcast across the 64 dh partitions with a
                        # single K=1 matmul ones[1,64].T @ recip_row[1,512].
                        # (Too DVE-heavy for the body, but the DVE is idle
                        # here and it kills the exposed roundtrip latency.)
                        seg = slice(h * S + qc * 512, h * S + (qc + 1) * 512)
                        nc.vector.reciprocal_approx_fast(
                            recip_row[0:1, seg], l_row[0:1, seg])
                        nc.vector.tensor_copy(
                            recip16_row[0:1, seg], recip_row[0:1, seg])
                        nc.tensor.matmul(
                            bc[:], ones1[0:1, :], recip16_row[0:1, seg],
                            start=True, stop=True)
                    else:
                        # body: 1/l on the [q-partition] transposed copy,
                        # broadcast over the 64 dh rows with K=128 matmuls
                        # against the identity (tiny DVE footprint)
                        c = slice(h * NT + 4 * qc, h * NT + 4 * qc + 4)
                        nc.vector.reciprocal(recipT[:, c], lT[:, c])
                        nc.vector.tensor_copy(recipT16[:, c], recipT[:, c])
                        for t4 in range(4):
                            col = h * NT + 4 * qc + t4
                            nc.tensor.matmul(
                                bc[:, t4 * 128:(t4 + 1) * 128],
                                recipT16[:, col:col + 1]
                                .to_broadcast((128, DH)),
                                ident[:],
                                start=True, stop=True)
                    nc.vector.tensor_mul(
                        outT[hp:hp + 64, hc, qc * 512:(qc + 1) * 512],
                        outT[hp:hp + 64, hc, qc * 512:(qc + 1) * 512],
                        bc[:])

                def wo_tile(qt, oc):
                    if True:
                        if True:
                            yps = ybcp.tile([128, 512], f32, tag="ybc",
                                            name=f"yps_{qt}_{oc}")
                            for fc in range(FC):
                                nc.tensor.matmul(
                                    yps[:],
                                    outT[:, fc, qt * 128:(qt + 1) * 128],
                                    wo[:, fc, oc * 512:(oc + 1) * 512],
                                    start=(fc == 0), stop=(fc == FC - 1))
                            ysb = ysb_pool.tile([128, 512], f16, tag="ysb",
                                                name=f"ysb_{qt}_{oc}")
                            # final chunk: alternate the psum->sbuf cast
                            # between Vector and Scalar so 8 casts don't
                            # serialize on one engine at the very end
                            if qt >= 4 * (NQ - 1) and oc == 1:
                                nc.scalar.copy(ysb[:], yps[:])
                            else:
                                nc.vector.tensor_copy(ysb[:], yps[:])
                            # y stores: sync hw queue (gpsimd software-DGE
                            # stores measured slower overall); final chunk
                            # alternates sync/scalar so the last 1MB drains
                            # 2-wide while both queues are idle
                            if qt >= 4 * (NQ - 1):
                                q = (nc.sync, nc.scalar)[(qt * 2 + oc) % 2]
                            else:
                                q = nc.sync
                            q.dma_start(
                                y_d[qt * 128:(qt + 1) * 128,
                                    oc * 512:(oc + 1) * 512],
                                ysb[:])

                # fc0 projections + v first so attention starts earliest;
                # fc1 q/k groups run as fillers inside att(0,0)
                for w_sb, dstT in ((wq, qT), (wk, kT)):
                    proj_qk_group(w_sb, dstT, 0, 0)
                proj_v(range(0, 4))
                for w_sb, dstT in ((wq, qT), (wk, kT)):
                    for part in range(2):
                        fillers.append(
                            lambda w=w_sb, d=dstT, p=part:
                            proj_qk_part(w, d, 1, 0, p))

                # Schedule (half-chunk lag): norm(qc,0) runs as a filler late
                # inside att(qc,1); norm(qc,1) + wo(qc) run inside att(qc+1,0).
                # Keeps every lT DMA roundtrip hidden by a full att phase and
                # leaves only norm(3,1)+wo(3) as the (short, HAM-kept-warm)
                # tail.  All filler units are ~0.4-0.8us of PE work so they
                # match the per-AV-group bubbles inside the blocks.
                for qc in range(NQ):
                    if qc >= 1:
                        for hp2 in range(2):
                            fillers.append(
                                lambda q=qc - 1, p=hp2: norm_half(q, 1, p))
                        for qt in range(4 * (qc - 1), 4 * qc):
                            for oc in range(2):
                                fillers.append(
                                    lambda a=qt, b=oc: wo_tile(a, b))
                    att_hc(qc, 0)
                    # keep 2 units back so the next att call's first blocks
                    # have filler work to cover their exp-pipeline refill
                    # (safe: the tail of the deque here is norm/wo units,
                    # which have no dependency on the next chunk's q/k)
                    run_filler(max(0, len(fillers) - (2 if qc >= 1 else 0)))
                    if qc + 1 < NQ:
                        for w_sb, dstT in ((wq, qT), (wk, kT)):
                            for fc in range(FC):
                                for part in range(2):
                                    fillers.append(
                                        lambda w=w_sb, d=dstT, f=fc,
                                        t=qc + 1, p=part:
                                        proj_qk_part(w, d, f, t, p))
                        for tt in range(4 * (qc + 1), 4 * (qc + 2)):
                            for part in range(2):
                                fillers.append(
                                    lambda t=tt, p=part: proj_v_part(t, p))
                    for hp2 in range(2):
                        fillers.append(
                            lambda q=qc, p=hp2: norm_half(q, 0, p))
                    att_hc(qc, 1)
                    # qc=0 must fully drain (qk(1) units pend and att(1,0)
                    # scores would deadlock behind them) and so must the
                    # last chunk (no more run_filler calls after the loop);
                    # middle chunks keep 2 norm units back for the next
                    # chunk-start bubble
                    keep = 2 if 1 <= qc < NQ - 1 else 0
                    run_filler(max(0, len(fillers) - keep))
                # tail: bridge the last lT roundtrip (~2us) with just enough
                # warm matmuls to hold the PE clock-gate at 8/8 without
                # delaying the final norm+wo work behind them
                warm_mm(8)
                norm_half(NQ - 1, 1, 0)
                norm_half(NQ - 1, 1, 1)
                for qt in range(4 * (NQ - 1), 4 * NQ):
                    for oc in range(2):
                        wo_tile(qt, oc)

            if dbg:
                nc.sync.dma_start(qT_dbg[:], qT[:])
                nc.sync.dma_start(kT_dbg[:], kT[:])
                nc.sync.dma_start(v_dbg[:], v[:])
                nc.sync.dma_start(outT_dbg[:], outT[:])
                nc.sync.dma_start(l_dbg[:], l_row[0:1, :])

    nc.compile()

    from concourse.bass_interp import get_hw_module
    nc.m = get_hw_module(nc.m)

    _CACHE[key] = nc
    return nc


def _make_mask():
    # mask[p, j] = 1 where (j - p) >= 384; slices of width 512 at offset
    # 384-128*r give the causal mask for a diagonal block at relative
    # position r (k block kb = 4*qc + r vs the 512-wide q chunk qc)
    j = np.arange(896)[None, :]
    p = np.arange(128)[:, None]
    return ((j - p) >= 384).astype(np.float16)


def kernel(x, wq, wk, wv, wo):
    x = np.asarray(x, dtype=np.float32)
    wq = np.asarray(wq, dtype=np.float32)
    wk = np.asarray(wk, dtype=np.float32)
    wv = np.asarray(wv, dtype=np.float32)
    wo = np.asarray(wo, dtype=np.float32)

    from concourse import bass_utils

    nc = _build_program()
    mask = _make_mask()

    in_maps = []
    for c in range(8):
        b = c // 4
        hg = c % 4
        fs = slice(hg * F, (hg + 1) * F)
        xT = np.ascontiguousarray(x[b].T).astype(np.float16).reshape(DC, 128, S)
        wqT = np.ascontiguousarray((wq[fs, :] * 0.125).T).astype(np.float16)
        wkT = np.ascontiguousarray(wk[fs, :].T).astype(np.float16)
        wvT = np.ascontiguousarray(wv[fs, :].T).astype(np.float16)
        woT = np.ascontiguousarray(wo[:, fs].T).astype(np.float16)
        in_maps.append({
            "xT": xT,
            "wqT": wqT.reshape(DC, 128, F),
            "wkT": wkT.reshape(DC, 128, F),
            "wvT": wvT.reshape(DC, 128, F),
            "woT": woT.reshape(FC, 128, D),
            "mask": mask,
            "ident": np.eye(128, dtype=np.float16),
        })

    res = bass_utils.run_bass_kernel_spmd(nc, in_maps, core_ids=list(range(8)))
    ys = [res.results[c]["y"].astype(np.float32) for c in range(8)]
    out = np.stack([ys[0] + ys[1] + ys[2] + ys[3],
                    ys[4] + ys[5] + ys[6] + ys[7]])
    return out

